# revision 23
# baseline (speedup 1.0000x reference)
"""Trainium2 Bass kernel for a dense transformer block (B=4, N=2048, C=768, H=12).

Sharding: 8 cores = 4 batches x 2 sequence halves (queries split; K/V duplicated
per batch pair, no collectives). Each core receives its batch rolled so its own
1024 query rows are rows 0:1023.

v2 dataflow (cost-model-driven):
- All projection/attention-value/MLP matmuls run in fp8e4m3 with DoubleRow perf
  mode (2 contraction k-tiles per instruction, 0.5 cycles/column = 4x fp32r).
  Weights are folded (LN gains), prescaled by 32 on the host, and cast to fp8;
  the 1/32 unscale + bias ride the PSUM->SBUF copies (or gelu's scale/bias).
- Scores stay bf16 (precision-sensitive); softmax exp runs on the Act engine in
  [128, 4, 512] PSUM tiles (2048-column calls) writing fp8 directly; the
  denominator rides the value matmul as a 65th ones-row of V.
- Per-query 1/den is applied via a PE ones-outer-product broadcast plus one DVE
  multiply per (head, chunk) - no transposes.
- LN uses Sqrt+DVE-reciprocal (no Ln/Exp table thrash); only 4 act-table loads.
- All DMA goes through SP HWDGE (Pool engine stays free for psum->sbuf copies).
"""

import numpy as np

B, N, C = 4, 2048, 768
H, DH = 12, 64
HID = 4 * C
SCALE = DH ** -0.5
EPS = 1e-5
WS = 32.0
IWS = 1.0 / WS

P = 128
CT = C // P          # 6
NT = N // P          # 16
NO = N // 2          # 1024 own rows
NOT_ = NO // P       # 8
HT = HID // P        # 24


def _build_bass():
    import concourse.bass as bass
    import concourse.tile as tile
    from concourse import bacc, mybir
    from concourse.masks import make_identity
    from concourse.alu_op_type import AluOpType as A

    F32 = mybir.dt.float32
    BF = mybir.dt.bfloat16
    F8 = mybir.dt.float8e4
    AF = mybir.ActivationFunctionType
    DR = mybir.MatmulPerfMode.DoubleRow

    nc = bacc.Bacc("TRN2", target_bir_lowering=False, num_swdge_queues=4)

    xb = nc.dram_tensor("xb", [N, C], BF, kind="ExternalInput")
    w_qkv8 = nc.dram_tensor("w_qkv8", [C, 3 * C], F8, kind="ExternalInput")
    w_proj8 = nc.dram_tensor("w_proj8", [DH, H * C], F8, kind="ExternalInput")
    w_fc18 = nc.dram_tensor("w_fc18", [C, HID], F8, kind="ExternalInput")
    w_fc28 = nc.dram_tensor("w_fc28", [HID, C], F8, kind="ExternalInput")
    qkv_bias = nc.dram_tensor("qkv_bias", [3 * C], F32, kind="ExternalInput")
    fc1_bias = nc.dram_tensor("fc1_bias", [HID], F32, kind="ExternalInput")
    b_proj_eff = nc.dram_tensor("b_proj_eff", [C], F32, kind="ExternalInput")
    b_fc2 = nc.dram_tensor("b_fc2", [C], F32, kind="ExternalInput")
    out = nc.dram_tensor("out", [NO, C], F32, kind="ExternalOutput")

    dma = nc.sync.dma_start

    with tile.TileContext(nc) as tc:
        consts = tc.alloc_tile_pool(name="consts", bufs=1)
        wpool = tc.alloc_tile_pool(name="wpool", bufs=1)
        big = tc.alloc_tile_pool(name="big", bufs=1)
        qkpool = tc.alloc_tile_pool(name="qkpool", bufs=2)
        work = tc.alloc_tile_pool(name="work", bufs=2)
        io = tc.alloc_tile_pool(name="io", bufs=6)
        estream = tc.alloc_tile_pool(name="estream", bufs=4)

        ident_bf = consts.tile([P, P], BF)
        make_identity(nc, ident_bf)
        ones_bf = consts.tile([P, DH], BF)
        nc.gpsimd.memset(ones_bf, 1.0)
        eps_t = consts.tile([P, 1], F32)
        nc.vector.memset(eps_t, EPS)
        qbT = consts.tile([P, 18], F32)
        dma(out=qbT, in_=qkv_bias[:].rearrange("(t p) -> p t", p=P))
        f1bT = consts.tile([P, HT], F32)
        dma(out=f1bT, in_=fc1_bias[:].rearrange("(t p) -> p t", p=P))
        bf2T = consts.tile([P, CT], F32)
        dma(out=bf2T, in_=b_fc2[:].rearrange("(t p) -> p t", p=P))
        bpT = consts.tile([P, CT], F32)
        dma(out=bpT, in_=b_proj_eff[:].rearrange("(t p) -> p t", p=P))

        Wqkv_s = wpool.tile([P, CT, 3 * C], F8)
        Wp_s = wpool.tile([DH, H, C], F8)
        Wfc1_s = wpool.tile([P, CT, HID], F8)
        Wfc2_s = wpool.tile([P, HT, C], F8)

        hT = big.tile([P, CT, N], F8)          # LN1(x)^T, fp8
        VW = 80  # V cols + ones + pad: dual-fp8 ldweights needs M % 16 == 0
        Vf8 = big.tile([P, NT, H, VW], F8)
        Yf8 = big.tile([DH, H, NO], F8)         # normalized y, proj rhs
        x2lnT = big.tile([P, CT, NO], F8)       # LN2(x2)^T
        x2 = big.tile([P, NOT_, C], BF)         # x + attn + b_proj

        nc.vector.memset(Vf8[:, :, :, DH:VW], 1.0)

        def ln_apply(src, dst_bf, csize):
            # dst = (src - mean) * rsqrt(var+eps); gains/biases are host-folded
            st = work.tile([P, 3, 6], F32, tag="ln_st", bufs=5)
            for s in range(3):
                nc.vector.bn_stats(out=st[:, s, :], in_=src[:, s * 256:(s + 1) * 256])
            mv = work.tile([P, 2], F32, tag="ln_mv", bufs=5)
            nc.vector.bn_aggr(out=mv, in_=st)
            sd = work.tile([P, 1], F32, tag="ln_sd", bufs=5)
            nc.scalar.activation(out=sd, in_=mv[:, 1:2], func=AF.Sqrt, bias=eps_t)
            r = work.tile([P, 1], F32, tag="ln_r", bufs=5)
            nc.vector.reciprocal(out=r, in_=sd)
            nc.gpsimd.tensor_scalar(out=dst_bf, in0=src, scalar1=mv[:, 0:1],
                                     scalar2=r, op0=A.subtract, op1=A.mult)

        # ---------------- Phase A: LN1 -> hT (fp8, transposed), V fused per tile
        # V weight columns load first (small); Q/K columns mid-loop.
        dma(out=Wqkv_s[:, :, 2 * C:],
            in_=w_qkv8[:, 2 * C:].rearrange("(t p) j -> p t j", p=P))
        with tc.tile_pool(name="ps_a", bufs=1, space="PSUM") as ps_a:
            for i in range(NT):
                x_t = io.tile([P, C], BF, tag="x")
                dma(out=x_t, in_=xb[i * P:(i + 1) * P, :])
                if i == 10:
                    dma(out=Wqkv_s[:, :, :2 * C],
                        in_=w_qkv8[:, :2 * C].rearrange("(t p) j -> p t j", p=P))
                z = work.tile([P, C], BF, tag="z", bufs=4)
                ln_apply(x_t, z, C)
                tp = ps_a.tile([P, C], BF, tag="tr", bufs=3)
                for t in range(CT):
                    nc.tensor.transpose(tp[:, t * P:(t + 1) * P],
                                        z[:, t * P:(t + 1) * P], ident_bf)
                nc.scalar.activation(
                    out=hT[:, :, i * P:(i + 1) * P],
                    in_=tp[:].rearrange("p (t n) -> p t n", t=CT), func=AF.Copy)
                # V for this key tile rides right behind its hT slice
                vps = ps_a.tile([P, C], F32, tag="v", bufs=2)
                for k2 in range(3):
                    for sl, c0 in ((512, 0), (256, 512)):
                        nc.tensor.matmul(
                            vps[:, c0:c0 + sl],
                            hT[:, 2 * k2:2 * k2 + 2, i * P:(i + 1) * P],
                            Wqkv_s[:, 2 * k2:2 * k2 + 2, 2 * C + c0:2 * C + c0 + sl],
                            start=(k2 == 0), stop=(k2 == 2), perf_mode=DR)
                if i % 2 == 0:
                    nc.vector.tensor_scalar(
                        out=Vf8[:, i, :, 0:DH],
                        in0=vps[:].rearrange("p (h d) -> p h d", h=H),
                        scalar1=IWS, scalar2=None, op0=A.mult)
                else:
                    nc.scalar.activation(
                        out=Vf8[:, i, :, 0:DH],
                        in_=vps[:].rearrange("p (h d) -> p h d", h=H),
                        func=AF.Copy, scale=IWS)
        dma(out=Wp_s, in_=w_proj8[:].rearrange("d (h j) -> d h j", h=H))
        dma(out=Wfc1_s, in_=w_fc18[:].rearrange("(t p) j -> p t j", p=P))
        dma(out=Wfc2_s, in_=w_fc28[:].rearrange("(t p) j -> p t j", p=P))

        qk_tiles = {}

        def qk_produce(hp, pool):
            # Q (own rows) and K (all rows) for head pair hp
            QT_s = qkpool.tile([P, NO], BF, tag="qt")
            for q2 in range(2):
                qps = pool.tile([P, 512], F32, tag="qk")
                for k2 in range(3):
                    nc.tensor.matmul(
                        qps,
                        Wqkv_s[:, 2 * k2:2 * k2 + 2, hp * P:(hp + 1) * P],
                        hT[:, 2 * k2:2 * k2 + 2, q2 * 512:(q2 + 1) * 512],
                        start=(k2 == 0), stop=(k2 == 2), perf_mode=DR)
                nc.vector.tensor_scalar(
                    out=QT_s[:, q2 * 512:(q2 + 1) * 512], in0=qps,
                    scalar1=IWS, scalar2=qbT[:, hp:hp + 1],
                    op0=A.mult, op1=A.add)
            KT_s = qkpool.tile([P, N], BF, tag="kt")
            for q4 in range(4):
                kps = pool.tile([P, 512], F32, tag="qk")
                for k2 in range(3):
                    nc.tensor.matmul(
                        kps,
                        Wqkv_s[:, 2 * k2:2 * k2 + 2, C + hp * P:C + (hp + 1) * P],
                        hT[:, 2 * k2:2 * k2 + 2, q4 * 512:(q4 + 1) * 512],
                        start=(k2 == 0), stop=(k2 == 2), perf_mode=DR)
                nc.vector.tensor_scalar(
                    out=KT_s[:, q4 * 512:(q4 + 1) * 512], in0=kps,
                    scalar1=IWS, scalar2=qbT[:, 6 + hp:7 + hp],
                    op0=A.mult, op1=A.add)
            qk_tiles[hp] = (QT_s, KT_s)

        # ---------------- Phase B: attention, head-sequential
        with tc.tile_pool(name="ps_b", bufs=1, space="PSUM") as ps_b:
            qk_produce(0, ps_b)
            for h in range(H):
                hp, sub = h // 2, h % 2
                base = sub * DH
                if sub == 0 and hp > 0:
                    qk_produce(hp, ps_b)
                QT_s, KT_s = qk_tiles[hp]
                for ch in range(2):
                    y = ps_b.tile([80, 512], F32, tag="y", bufs=2)
                    for g in range(8):
                        S = ps_b.tile([P, 2, 512], F32, tag="s", bufs=2)
                        for j in range(2):
                            m = 2 * g + j
                            nc.tensor.matmul(
                                S[:, j, :],
                                KT_s[base:base + DH, m * P:(m + 1) * P],
                                QT_s[base:base + DH, ch * 512:(ch + 1) * 512],
                                start=True, stop=True)
                        E8 = estream.tile([P, 2, 512], F8, tag="e")
                        nc.scalar.activation(out=E8, in_=S, func=AF.Exp, scale=SCALE)
                        nc.tensor.matmul(
                            y, Vf8[:, 2 * g:2 * g + 2, h, :], E8,
                            start=(g == 0), stop=(g == 7), perf_mode=DR)
                    # normalize: rinv broadcast via PE ones outer product
                    rin = work.tile([P, 512], BF, tag="ri")
                    with nc.allow_low_precision(reason="bf16 softmax denom recip"):
                        nc.vector.reciprocal(out=rin[DH:DH + 1, :], in_=y[DH:DH + 1, :])
                    rb = ps_b.tile([DH, 512], F32, tag="rb")
                    nc.tensor.matmul(rb, ones_bf[DH:DH + 1, 0:DH],
                                     rin[DH:DH + 1, :], start=True, stop=True)
                    rbs = work.tile([DH, 512], BF, tag="rb")
                    nc.vector.tensor_copy(out=rbs, in_=rb)
                    nc.vector.tensor_tensor(
                        out=Yf8[:, h, ch * 512:(ch + 1) * 512],
                        in0=y[0:DH, :], in1=rbs, op=A.mult)

        # ---------------- Phase C: proj -> + x + b_proj -> x2
        with tc.tile_pool(name="ps_c", bufs=1, space="PSUM") as ps_c:
            for th in range(2):
                attnT = work.tile([P, CT, 512], BF, tag="at")
                for oc in range(CT):
                    pps = ps_c.tile([P, 512], F32, tag="p", bufs=2)
                    for j in range(CT):
                        nc.tensor.matmul(
                            pps, Wp_s[:, 2 * j:2 * j + 2, oc * P:(oc + 1) * P],
                            Yf8[:, 2 * j:2 * j + 2, th * 512:(th + 1) * 512],
                            start=(j == 0), stop=(j == CT - 1), perf_mode=DR)
                    nc.vector.tensor_scalar(out=attnT[:, oc, :], in0=pps,
                                            scalar1=IWS, scalar2=bpT[:, oc:oc + 1],
                                            op0=A.mult, op1=A.add)
                for i in range(4):
                    it = th * 4 + i
                    tpc = ps_c.tile([P, C], BF, tag="tr", bufs=3)
                    for t in range(CT):
                        nc.tensor.transpose(tpc[:, t * P:(t + 1) * P],
                                            attnT[:, t, i * P:(i + 1) * P], ident_bf)
                    x_t = io.tile([P, C], BF, tag="x")
                    dma(out=x_t, in_=xb[it * P:(it + 1) * P, :])
                    nc.vector.tensor_tensor(out=x2[:, it, :], in0=tpc, in1=x_t,
                                            op=A.add)

        # ---------------- Phase D: LN2 -> x2lnT
        with tc.tile_pool(name="ps_d", bufs=2, space="PSUM") as ps_d:
            for i in range(NOT_):
                z2 = work.tile([P, C], BF, tag="z", bufs=4)
                ln_apply(x2[:, i, :], z2, C)
                tpd = ps_d.tile([P, C], BF, tag="tr", bufs=3)
                for t in range(CT):
                    nc.tensor.transpose(tpd[:, t * P:(t + 1) * P],
                                        z2[:, t * P:(t + 1) * P], ident_bf)
                nc.scalar.activation(
                    out=x2lnT[:, :, i * P:(i + 1) * P],
                    in_=tpd[:].rearrange("p (t n) -> p t n", t=CT), func=AF.Copy)

        # ---------------- Phase E: MLP + residual -> out, per 512-token half
        with tc.tile_pool(name="ps_e", bufs=1, space="PSUM") as ps_e:
            for th in range(2):
                sl = slice(th * 512, (th + 1) * 512)
                GA8 = big.tile([P, HT, 512], F8, tag="ga", bufs=2)
                for hg in range(HT):
                    f1 = ps_e.tile([P, 512], F32, tag="f1", bufs=2)
                    for k2 in range(3):
                        nc.tensor.matmul(
                            f1, Wfc1_s[:, 2 * k2:2 * k2 + 2, hg * P:(hg + 1) * P],
                            x2lnT[:, 2 * k2:2 * k2 + 2, sl],
                            start=(k2 == 0), stop=(k2 == 2), perf_mode=DR)
                    nc.scalar.activation(out=GA8[:, hg, :], in_=f1, func=AF.Gelu,
                                         bias=f1bT[:, hg:hg + 1], scale=IWS)
                mlpT = work.tile([P, CT, 512], BF, tag="at")
                for oc in range(CT):
                    fa = ps_e.tile([P, 512], F32, tag="fa", bufs=2)
                    for j in range(12):
                        nc.tensor.matmul(
                            fa, Wfc2_s[:, 2 * j:2 * j + 2, oc * P:(oc + 1) * P],
                            GA8[:, 2 * j:2 * j + 2, :],
                            start=(j == 0), stop=(j == 11), perf_mode=DR)
                    nc.vector.tensor_scalar(out=mlpT[:, oc, :], in0=fa,
                                            scalar1=IWS, scalar2=bf2T[:, oc:oc + 1],
                                            op0=A.mult, op1=A.add)
                for i in range(4):
                    it = th * 4 + i
                    tpe = ps_e.tile([P, C], BF, tag="tr", bufs=3)
                    for t in range(CT):
                        nc.tensor.transpose(tpe[:, t * P:(t + 1) * P],
                                            mlpT[:, t, i * P:(i + 1) * P], ident_bf)
                    o_sb = work.tile([P, C], F32, tag="o")
                    nc.vector.tensor_tensor(out=o_sb, in0=tpe, in1=x2[:, it, :],
                                            op=A.add)
                    dma(out=out[it * P:(it + 1) * P, :], in_=o_sb)

        estream.release()
        io.release()
        work.release()
        qkpool.release()
        big.release()
        wpool.release()
        consts.release()

    nc.compile()
    return nc


_NC_CACHE = None


def kernel(x, ln1_g, ln1_b, w_qkv, w_proj, b_proj, ln2_g, ln2_b,
           w_fc1, b_fc1, w_fc2, b_fc2):
    global _NC_CACHE
    import ml_dtypes
    from concourse.bass_utils import run_bass_kernel_spmd

    F8NP = ml_dtypes.float8_e4m3
    BFNP = ml_dtypes.bfloat16

    x = np.asarray(x, np.float32)
    ln1_g = np.asarray(ln1_g, np.float32)
    ln1_b = np.asarray(ln1_b, np.float32)
    ln2_g = np.asarray(ln2_g, np.float32)
    ln2_b = np.asarray(ln2_b, np.float32)
    w_qkv = np.asarray(w_qkv, np.float32)
    w_proj = np.asarray(w_proj, np.float32)
    w_fc1 = np.asarray(w_fc1, np.float32)
    w_fc2 = np.asarray(w_fc2, np.float32)

    # host-side folding + fp8 prescaling
    w_qkv8 = np.asarray(w_qkv * ln1_g[:, None] * WS, F8NP)
    qkv_bias = (ln1_b @ w_qkv).astype(np.float32)
    # proj weights rearranged [DH, H, C] so head pairs share partitions 0:64
    w_proj8 = np.ascontiguousarray(
        np.asarray(w_proj * WS, F8NP).reshape(H, DH, C).transpose(1, 0, 2)
    ).reshape(DH, H * C)
    w_fc18 = np.asarray(w_fc1 * ln2_g[:, None] * WS, F8NP)
    fc1_bias = (ln2_b @ w_fc1 + np.asarray(b_fc1, np.float32)).astype(np.float32)
    w_fc28 = np.asarray(w_fc2 * WS, F8NP)

    shared = {
        "w_qkv8": w_qkv8,
        "w_proj8": w_proj8,
        "w_fc18": w_fc18,
        "w_fc28": w_fc28,
        "qkv_bias": qkv_bias,
        "fc1_bias": fc1_bias,
        "b_proj_eff": (np.asarray(b_proj, np.float32)
                       + qkv_bias[2 * C:] @ w_proj).astype(np.float32),
        "b_fc2": np.asarray(b_fc2, np.float32),
    }
    in_maps = []
    for c in range(8):
        b, hh = c // 2, c % 2
        xbv = np.ascontiguousarray(
            np.asarray(np.roll(x[b], -hh * NO, axis=0), BFNP))
        in_maps.append({"xb": xbv, **shared})

    if _NC_CACHE is None:
        _NC_CACHE = _build_bass()
    res = run_bass_kernel_spmd(_NC_CACHE, in_maps, core_ids=list(range(8)))

    outp = np.empty((B, N, C), np.float32)
    for c in range(8):
        b, hh = c // 2, c % 2
        outp[b, hh * NO:(hh + 1) * NO, :] = res.results[c]["out"]
    return outp


# revision 24
# speedup vs baseline: 1.0059x; 1.0059x over previous
"""Trainium2 Bass kernel for a dense transformer block (B=4, N=2048, C=768, H=12).

Sharding: 8 cores = 4 batches x 2 sequence halves (queries split; K/V duplicated
per batch pair, no collectives). Each core receives its batch rolled so its own
1024 query rows are rows 0:1023.

v2 dataflow (cost-model-driven):
- All projection/attention-value/MLP matmuls run in fp8e4m3 with DoubleRow perf
  mode (2 contraction k-tiles per instruction, 0.5 cycles/column = 4x fp32r).
  Weights are folded (LN gains), prescaled by 32 on the host, and cast to fp8;
  the 1/32 unscale + bias ride the PSUM->SBUF copies (or gelu's scale/bias).
- Scores stay bf16 (precision-sensitive); softmax exp runs on the Act engine in
  [128, 4, 512] PSUM tiles (2048-column calls) writing fp8 directly; the
  denominator rides the value matmul as a 65th ones-row of V.
- Per-query 1/den is applied via a PE ones-outer-product broadcast plus one DVE
  multiply per (head, chunk) - no transposes.
- LN uses Sqrt+DVE-reciprocal (no Ln/Exp table thrash); only 4 act-table loads.
- All DMA goes through SP HWDGE (Pool engine stays free for psum->sbuf copies).
"""

import numpy as np

B, N, C = 4, 2048, 768
H, DH = 12, 64
HID = 4 * C
SCALE = DH ** -0.5
EPS = 1e-5
WS = 32.0
IWS = 1.0 / WS

P = 128
CT = C // P          # 6
NT = N // P          # 16
NO = N // 2          # 1024 own rows
NOT_ = NO // P       # 8
HT = HID // P        # 24


def _build_bass():
    import concourse.bass as bass
    import concourse.tile as tile
    from concourse import bacc, mybir
    from concourse.masks import make_identity
    from concourse.alu_op_type import AluOpType as A

    F32 = mybir.dt.float32
    BF = mybir.dt.bfloat16
    F8 = mybir.dt.float8e4
    AF = mybir.ActivationFunctionType
    DR = mybir.MatmulPerfMode.DoubleRow

    nc = bacc.Bacc("TRN2", target_bir_lowering=False, num_swdge_queues=4)

    xb = nc.dram_tensor("xb", [N, C], BF, kind="ExternalInput")
    w_qkv8 = nc.dram_tensor("w_qkv8", [C, 3 * C], F8, kind="ExternalInput")
    w_proj8 = nc.dram_tensor("w_proj8", [DH, H * C], F8, kind="ExternalInput")
    w_fc18 = nc.dram_tensor("w_fc18", [C, HID], F8, kind="ExternalInput")
    w_fc28 = nc.dram_tensor("w_fc28", [HID, C], F8, kind="ExternalInput")
    qkv_bias = nc.dram_tensor("qkv_bias", [3 * C], F32, kind="ExternalInput")
    fc1_bias = nc.dram_tensor("fc1_bias", [HID], F32, kind="ExternalInput")
    b_proj_eff = nc.dram_tensor("b_proj_eff", [C], F32, kind="ExternalInput")
    b_fc2 = nc.dram_tensor("b_fc2", [C], F32, kind="ExternalInput")
    out = nc.dram_tensor("out", [NO, C], F32, kind="ExternalOutput")

    dma = nc.sync.dma_start

    with tile.TileContext(nc) as tc:
        consts = tc.alloc_tile_pool(name="consts", bufs=1)
        wpool = tc.alloc_tile_pool(name="wpool", bufs=1)
        big = tc.alloc_tile_pool(name="big", bufs=1)
        qkpool = tc.alloc_tile_pool(name="qkpool", bufs=2)
        work = tc.alloc_tile_pool(name="work", bufs=2)
        io = tc.alloc_tile_pool(name="io", bufs=6)
        estream = tc.alloc_tile_pool(name="estream", bufs=3)

        ident_bf = consts.tile([P, P], BF)
        make_identity(nc, ident_bf)
        ones_bf = consts.tile([P, DH], BF)
        nc.gpsimd.memset(ones_bf, 1.0)
        eps_t = consts.tile([P, 1], F32)
        nc.vector.memset(eps_t, EPS)
        qbT = consts.tile([P, 18], F32)
        dma(out=qbT, in_=qkv_bias[:].rearrange("(t p) -> p t", p=P))
        f1bT = consts.tile([P, HT], F32)
        dma(out=f1bT, in_=fc1_bias[:].rearrange("(t p) -> p t", p=P))
        bf2T = consts.tile([P, CT], F32)
        dma(out=bf2T, in_=b_fc2[:].rearrange("(t p) -> p t", p=P))
        bpT = consts.tile([P, CT], F32)
        dma(out=bpT, in_=b_proj_eff[:].rearrange("(t p) -> p t", p=P))

        Wqkv_s = wpool.tile([P, CT, 3 * C], F8)
        Wp_s = wpool.tile([DH, H, C], F8)
        Wfc1_s = wpool.tile([P, CT, HID], F8)
        Wfc2_s = wpool.tile([P, HT, C], F8)

        hT = big.tile([P, CT, N], F8)          # LN1(x)^T, fp8
        VW = 80  # V cols + ones + pad: dual-fp8 ldweights needs M % 16 == 0
        Vf8 = big.tile([P, NT, H, VW], F8)
        Yf8 = big.tile([DH, H, NO], F8)         # normalized y, proj rhs
        x2lnT = big.tile([P, CT, NO], F8)       # LN2(x2)^T
        x2 = big.tile([P, NOT_, C], BF)         # x + attn + b_proj

        nc.vector.memset(Vf8[:, :, :, DH:VW], 1.0)

        def ln_apply(src, dst_bf, csize):
            # dst = (src - mean) * rsqrt(var+eps); gains/biases are host-folded
            st = work.tile([P, 3, 6], F32, tag="ln_st", bufs=5)
            for s in range(3):
                nc.vector.bn_stats(out=st[:, s, :], in_=src[:, s * 256:(s + 1) * 256])
            mv = work.tile([P, 2], F32, tag="ln_mv", bufs=5)
            nc.vector.bn_aggr(out=mv, in_=st)
            sd = work.tile([P, 1], F32, tag="ln_sd", bufs=5)
            nc.scalar.activation(out=sd, in_=mv[:, 1:2], func=AF.Sqrt, bias=eps_t)
            r = work.tile([P, 1], F32, tag="ln_r", bufs=5)
            nc.vector.reciprocal(out=r, in_=sd)
            nc.gpsimd.tensor_scalar(out=dst_bf, in0=src, scalar1=mv[:, 0:1],
                                     scalar2=r, op0=A.subtract, op1=A.mult)

        # ---------------- Phase A: LN1 -> hT (fp8, transposed), V fused per tile
        # V weight columns load first (small); Q/K columns mid-loop.
        dma(out=Wqkv_s[:, :, 2 * C:],
            in_=w_qkv8[:, 2 * C:].rearrange("(t p) j -> p t j", p=P))
        with tc.tile_pool(name="ps_a", bufs=1, space="PSUM") as ps_a:
            for i in range(NT):
                x_t = io.tile([P, C], BF, tag="x")
                dma(out=x_t, in_=xb[i * P:(i + 1) * P, :])
                if i == 10:
                    dma(out=Wqkv_s[:, :, :2 * C],
                        in_=w_qkv8[:, :2 * C].rearrange("(t p) j -> p t j", p=P))
                z = work.tile([P, C], BF, tag="z", bufs=4)
                ln_apply(x_t, z, C)
                tp = ps_a.tile([P, C], BF, tag="tr", bufs=3)
                for t in range(CT):
                    nc.tensor.transpose(tp[:, t * P:(t + 1) * P],
                                        z[:, t * P:(t + 1) * P], ident_bf)
                nc.scalar.activation(
                    out=hT[:, :, i * P:(i + 1) * P],
                    in_=tp[:].rearrange("p (t n) -> p t n", t=CT), func=AF.Copy)
                # V for this key tile rides right behind its hT slice
                vps = ps_a.tile([P, C], F32, tag="v", bufs=2)
                for k2 in range(3):
                    for sl, c0 in ((512, 0), (256, 512)):
                        nc.tensor.matmul(
                            vps[:, c0:c0 + sl],
                            hT[:, 2 * k2:2 * k2 + 2, i * P:(i + 1) * P],
                            Wqkv_s[:, 2 * k2:2 * k2 + 2, 2 * C + c0:2 * C + c0 + sl],
                            start=(k2 == 0), stop=(k2 == 2), perf_mode=DR)
                if i % 2 == 0:
                    nc.vector.tensor_scalar(
                        out=Vf8[:, i, :, 0:DH],
                        in0=vps[:].rearrange("p (h d) -> p h d", h=H),
                        scalar1=IWS, scalar2=None, op0=A.mult)
                else:
                    nc.scalar.activation(
                        out=Vf8[:, i, :, 0:DH],
                        in_=vps[:].rearrange("p (h d) -> p h d", h=H),
                        func=AF.Copy, scale=IWS)
        dma(out=Wp_s, in_=w_proj8[:].rearrange("d (h j) -> d h j", h=H))
        dma(out=Wfc1_s, in_=w_fc18[:].rearrange("(t p) j -> p t j", p=P))
        dma(out=Wfc2_s, in_=w_fc28[:].rearrange("(t p) j -> p t j", p=P))

        qk_tiles = {}

        def qk_produce(hp, pool):
            # Q (own rows) and K (all rows) for head pair hp
            QT_s = qkpool.tile([P, NO], BF, tag="qt")
            for q2 in range(2):
                qps = pool.tile([P, 512], F32, tag="qk")
                for k2 in range(3):
                    nc.tensor.matmul(
                        qps,
                        Wqkv_s[:, 2 * k2:2 * k2 + 2, hp * P:(hp + 1) * P],
                        hT[:, 2 * k2:2 * k2 + 2, q2 * 512:(q2 + 1) * 512],
                        start=(k2 == 0), stop=(k2 == 2), perf_mode=DR)
                nc.vector.tensor_scalar(
                    out=QT_s[:, q2 * 512:(q2 + 1) * 512], in0=qps,
                    scalar1=IWS, scalar2=qbT[:, hp:hp + 1],
                    op0=A.mult, op1=A.add)
            KT_s = qkpool.tile([P, N], BF, tag="kt")
            for q4 in range(4):
                kps = pool.tile([P, 512], F32, tag="qk")
                for k2 in range(3):
                    nc.tensor.matmul(
                        kps,
                        Wqkv_s[:, 2 * k2:2 * k2 + 2, C + hp * P:C + (hp + 1) * P],
                        hT[:, 2 * k2:2 * k2 + 2, q4 * 512:(q4 + 1) * 512],
                        start=(k2 == 0), stop=(k2 == 2), perf_mode=DR)
                nc.vector.tensor_scalar(
                    out=KT_s[:, q4 * 512:(q4 + 1) * 512], in0=kps,
                    scalar1=IWS, scalar2=qbT[:, 6 + hp:7 + hp],
                    op0=A.mult, op1=A.add)
            qk_tiles[hp] = (QT_s, KT_s)

        # ---------------- Phase B: attention, head-sequential
        with tc.tile_pool(name="ps_b", bufs=1, space="PSUM") as ps_b:
            qk_produce(0, ps_b)
            for h in range(H):
                hp, sub = h // 2, h % 2
                base = sub * DH
                if sub == 0 and hp > 0:
                    qk_produce(hp, ps_b)
                QT_s, KT_s = qk_tiles[hp]
                for ch in range(2):
                    y = ps_b.tile([80, 512], F32, tag="y", bufs=2)
                    for g in range(8):
                        S = ps_b.tile([P, 2, 512], F32, tag="s", bufs=2)
                        for j in range(2):
                            m = 2 * g + j
                            nc.tensor.matmul(
                                S[:, j, :],
                                KT_s[base:base + DH, m * P:(m + 1) * P],
                                QT_s[base:base + DH, ch * 512:(ch + 1) * 512],
                                start=True, stop=True)
                        E8 = estream.tile([P, 2, 512], F8, tag="e")
                        nc.scalar.activation(out=E8, in_=S, func=AF.Exp, scale=SCALE)
                        nc.tensor.matmul(
                            y, Vf8[:, 2 * g:2 * g + 2, h, :], E8,
                            start=(g == 0), stop=(g == 7), perf_mode=DR)
                    # normalize: rinv broadcast via PE ones outer product
                    rin = work.tile([P, 512], BF, tag="ri")
                    with nc.allow_low_precision(reason="bf16 softmax denom recip"):
                        nc.vector.reciprocal(out=rin[DH:DH + 1, :], in_=y[DH:DH + 1, :])
                    rb = ps_b.tile([DH, 512], F32, tag="rb")
                    nc.tensor.matmul(rb, ones_bf[DH:DH + 1, 0:DH],
                                     rin[DH:DH + 1, :], start=True, stop=True)
                    rbs = work.tile([DH, 512], BF, tag="rb")
                    nc.vector.tensor_copy(out=rbs, in_=rb)
                    nc.vector.tensor_tensor(
                        out=Yf8[:, h, ch * 512:(ch + 1) * 512],
                        in0=y[0:DH, :], in1=rbs, op=A.mult)

        # ---------------- Phase C: proj -> + x + b_proj -> x2
        with tc.tile_pool(name="ps_c", bufs=1, space="PSUM") as ps_c:
            for th in range(2):
                attnT = work.tile([P, CT, 512], BF, tag="at")
                for oc in range(CT):
                    pps = ps_c.tile([P, 512], F32, tag="p", bufs=2)
                    for j in range(CT):
                        nc.tensor.matmul(
                            pps, Wp_s[:, 2 * j:2 * j + 2, oc * P:(oc + 1) * P],
                            Yf8[:, 2 * j:2 * j + 2, th * 512:(th + 1) * 512],
                            start=(j == 0), stop=(j == CT - 1), perf_mode=DR)
                    nc.vector.tensor_scalar(out=attnT[:, oc, :], in0=pps,
                                            scalar1=IWS, scalar2=bpT[:, oc:oc + 1],
                                            op0=A.mult, op1=A.add)
                for i in range(4):
                    it = th * 4 + i
                    tpc = ps_c.tile([P, C], BF, tag="tr", bufs=2)
                    for t in range(CT):
                        nc.tensor.transpose(tpc[:, t * P:(t + 1) * P],
                                            attnT[:, t, i * P:(i + 1) * P], ident_bf)
                    x_t = io.tile([P, C], BF, tag="x")
                    dma(out=x_t, in_=xb[it * P:(it + 1) * P, :])
                    nc.vector.tensor_tensor(out=x2[:, it, :], in0=tpc, in1=x_t,
                                            op=A.add)

        # ---------------- Phase D: LN2 -> x2lnT
        with tc.tile_pool(name="ps_d", bufs=2, space="PSUM") as ps_d:
            for i in range(NOT_):
                z2 = work.tile([P, C], BF, tag="z", bufs=4)
                ln_apply(x2[:, i, :], z2, C)
                tpd = ps_d.tile([P, C], BF, tag="tr")
                for t in range(CT):
                    nc.tensor.transpose(tpd[:, t * P:(t + 1) * P],
                                        z2[:, t * P:(t + 1) * P], ident_bf)
                nc.scalar.activation(
                    out=x2lnT[:, :, i * P:(i + 1) * P],
                    in_=tpd[:].rearrange("p (t n) -> p t n", t=CT), func=AF.Copy)

        # ---------------- Phase E: MLP + residual -> out, per 512-token half
        with tc.tile_pool(name="ps_e", bufs=1, space="PSUM") as ps_e:
            for th in range(2):
                sl = slice(th * 512, (th + 1) * 512)
                GA8 = big.tile([P, HT, 512], F8, tag="ga", bufs=2)
                for hg in range(HT):
                    f1 = ps_e.tile([P, 512], F32, tag="f1", bufs=2)
                    for k2 in range(3):
                        nc.tensor.matmul(
                            f1, Wfc1_s[:, 2 * k2:2 * k2 + 2, hg * P:(hg + 1) * P],
                            x2lnT[:, 2 * k2:2 * k2 + 2, sl],
                            start=(k2 == 0), stop=(k2 == 2), perf_mode=DR)
                    nc.scalar.activation(out=GA8[:, hg, :], in_=f1, func=AF.Gelu,
                                         bias=f1bT[:, hg:hg + 1], scale=IWS)
                mlpT = work.tile([P, CT, 512], BF, tag="at")
                for oc in range(CT):
                    fa = ps_e.tile([P, 512], F32, tag="fa", bufs=2)
                    for j in range(12):
                        nc.tensor.matmul(
                            fa, Wfc2_s[:, 2 * j:2 * j + 2, oc * P:(oc + 1) * P],
                            GA8[:, 2 * j:2 * j + 2, :],
                            start=(j == 0), stop=(j == 11), perf_mode=DR)
                    nc.vector.tensor_scalar(out=mlpT[:, oc, :], in0=fa,
                                            scalar1=IWS, scalar2=bf2T[:, oc:oc + 1],
                                            op0=A.mult, op1=A.add)
                for i in range(4):
                    it = th * 4 + i
                    tpe = ps_e.tile([P, C], BF, tag="tr", bufs=2)
                    for t in range(CT):
                        nc.tensor.transpose(tpe[:, t * P:(t + 1) * P],
                                            mlpT[:, t, i * P:(i + 1) * P], ident_bf)
                    o_sb = work.tile([P, C], F32, tag="o")
                    nc.vector.tensor_tensor(out=o_sb, in0=tpe, in1=x2[:, it, :],
                                            op=A.add)
                    dma(out=out[it * P:(it + 1) * P, :], in_=o_sb)

        estream.release()
        io.release()
        work.release()
        qkpool.release()
        big.release()
        wpool.release()
        consts.release()

    nc.compile()
    return nc


_NC_CACHE = None


def kernel(x, ln1_g, ln1_b, w_qkv, w_proj, b_proj, ln2_g, ln2_b,
           w_fc1, b_fc1, w_fc2, b_fc2):
    global _NC_CACHE
    import ml_dtypes
    from concourse.bass_utils import run_bass_kernel_spmd

    F8NP = ml_dtypes.float8_e4m3
    BFNP = ml_dtypes.bfloat16

    x = np.asarray(x, np.float32)
    ln1_g = np.asarray(ln1_g, np.float32)
    ln1_b = np.asarray(ln1_b, np.float32)
    ln2_g = np.asarray(ln2_g, np.float32)
    ln2_b = np.asarray(ln2_b, np.float32)
    w_qkv = np.asarray(w_qkv, np.float32)
    w_proj = np.asarray(w_proj, np.float32)
    w_fc1 = np.asarray(w_fc1, np.float32)
    w_fc2 = np.asarray(w_fc2, np.float32)

    # host-side folding + fp8 prescaling
    w_qkv8 = np.asarray(w_qkv * ln1_g[:, None] * WS, F8NP)
    qkv_bias = (ln1_b @ w_qkv).astype(np.float32)
    # proj weights rearranged [DH, H, C] so head pairs share partitions 0:64
    w_proj8 = np.ascontiguousarray(
        np.asarray(w_proj * WS, F8NP).reshape(H, DH, C).transpose(1, 0, 2)
    ).reshape(DH, H * C)
    w_fc18 = np.asarray(w_fc1 * ln2_g[:, None] * WS, F8NP)
    fc1_bias = (ln2_b @ w_fc1 + np.asarray(b_fc1, np.float32)).astype(np.float32)
    w_fc28 = np.asarray(w_fc2 * WS, F8NP)

    shared = {
        "w_qkv8": w_qkv8,
        "w_proj8": w_proj8,
        "w_fc18": w_fc18,
        "w_fc28": w_fc28,
        "qkv_bias": qkv_bias,
        "fc1_bias": fc1_bias,
        "b_proj_eff": (np.asarray(b_proj, np.float32)
                       + qkv_bias[2 * C:] @ w_proj).astype(np.float32),
        "b_fc2": np.asarray(b_fc2, np.float32),
    }
    in_maps = []
    for c in range(8):
        b, hh = c // 2, c % 2
        xbv = np.ascontiguousarray(
            np.asarray(np.roll(x[b], -hh * NO, axis=0), BFNP))
        in_maps.append({"xb": xbv, **shared})

    if _NC_CACHE is None:
        _NC_CACHE = _build_bass()
    res = run_bass_kernel_spmd(_NC_CACHE, in_maps, core_ids=list(range(8)))

    outp = np.empty((B, N, C), np.float32)
    for c in range(8):
        b, hh = c // 2, c % 2
        outp[b, hh * NO:(hh + 1) * NO, :] = res.results[c]["out"]
    return outp
